# revision 23
# baseline (speedup 1.0000x reference)
"""Locally-connected (masked linear) layer for 8 TRN2 NeuronCores.

y = x @ (W * M)^T + b
  x: [4096, 4096] f32, W/M: [4096, 4096] f32, b: [4096] f32.

Strategy (2D: 4-way over out_features x 2-way over batch):
  - Core c owns out rows [ob*1024, (ob+1)*1024) (ob = c>>1) and batch
    rows [bb*2048, (bb+1)*2048) (bb = c&1).  This halves the x stream
    per batch group vs pure tensor-parallel, so weight streaming
    during the first group fits far under the HBM budget.
  - M is a static binary neighborhood filter, so the host folds it
    into the weights while laying them out (masked_w = W*M in bf16,
    exact since M is {0,1}); the device streams masked weight slabs
    straight into their resident SBUF tiles.
  - Host uploads contraction-major slabs: two 1-k-tile head weight
    slabs (so the PE can start early) then 2-k-tile slabs
    ([128, 2*1024] bf16, 4KB contiguous rows), and x in 4-k-tile
    blocks [128, 4*512] per batch group.  Weights issue on the SP
    queue; x rides the Activation queue except group-0 blocks >=2,
    which interleave into the SP queue behind the weight slabs that
    gate the PE.  Large contiguous DMAs keep the ~650ns/DMA
    descriptor-generation cost off the critical path.
  - A handful of warmup matmuls on a zeroed tile burn the PE DVFS
    ramp during the initial DMA wait so real matmuls start at full
    clock.
  - PE: y^T[j] += mw[k]^T.T @ x[k] accumulated over 32 k-tiles in 8
    fp32 PSUM banks per batch group; with oc=8 the next group's bank
    reuse hides under the current group's tail.
  - Evacuation adds bias per-partition and casts to bf16, split
    between DVE (j 0-3) and the Activation engine (j 4-7) in
    quarter-group DMAs so the last group's transfers overlap the
    evac chain.
  - Host reassembles the 8 y^T blocks, transposes, casts to f32.
"""

import os

import numpy as np
import ml_dtypes

BATCH = 4096
IN_F = 4096
OUT_F = 4096
N_CORES = 8
N_OB = 4                    # out-feature blocks
N_BB = 2                    # batch blocks
O_SHARD = OUT_F // N_OB     # 1024 out rows per core
B_SHARD = BATCH // N_BB     # 2048 batch rows per core
P = 128                     # SBUF partitions
BG = 512                    # batch columns per PSUM accumulation group
SK = 2                      # k-tiles per main weight slab
HEAD = 2                    # leading 1-k-tile weight slabs
XSK = 4                     # k-tiles per x slab
N_WARM = 7                  # PE warmup matmuls

_BF16 = ml_dtypes.bfloat16
_NC = None
LAST_RESULT = None


def _ensure_axon_hooks_stub():
    """bass_utils' axon trace path imports antenv.axon_hooks, which this
    container's antenv stub lacks. Install a minimal registry so the
    import succeeds (hook None => bass_utils skips tracing gracefully)."""
    import sys
    import types

    try:
        import antenv.axon_hooks  # noqa: F401
        return
    except ImportError:
        pass
    import antenv

    mod = types.ModuleType("antenv.axon_hooks")
    mod._HOOK = None

    def set_axon_ntff_profile_hook(h):
        mod._HOOK = h

    def get_axon_ntff_profile_hook():
        return mod._HOOK

    mod.set_axon_ntff_profile_hook = set_axon_ntff_profile_hook
    mod.get_axon_ntff_profile_hook = get_axon_ntff_profile_hook
    antenv.axon_hooks = mod
    sys.modules["antenv.axon_hooks"] = mod


def _install_real_ntff_hook():
    """Wire the ctypes NTFF profiling hook (normally registered by the
    boot middleware) so run_bass_kernel_spmd(trace=True) works."""
    _ensure_axon_hooks_stub()
    import antenv.axon_hooks as ah

    if ah.get_axon_ntff_profile_hook() is None:
        try:
            from trn_agent_boot.trn_boot import _ntff_profile_via_ctypes

            hook = _ntff_profile_via_ctypes("/opt/axon/libaxon_pjrt.so")
            if hook is not None:
                ah.set_axon_ntff_profile_hook(hook)
        except Exception:
            pass
    try:
        import concourse.bass_utils as bu

        bu.upload_artifacts = lambda tmpdir: "local://" + str(tmpdir)
    except Exception:
        pass


def wm_slab_sizes(kt, head=HEAD, sk=SK):
    """k-tile counts per weight slab: HEAD leading singles, then SK-wide."""
    assert kt > head and (kt - head) % sk == 0
    return [1] * head + [sk] * ((kt - head) // sk)


def slab_weights(wT, sk):
    """[k*P, o] contraction-major -> [nslab*P, sk*o] slab layout."""
    kp, o = wT.shape
    ns = kp // P // sk
    return np.ascontiguousarray(
        wT.reshape(ns, sk, P, o).transpose(0, 2, 1, 3).reshape(ns * P, sk * o))


def slab_x(xT, bg=BG, sk=XSK):
    """[in_f, batch] -> [ng*nslab*P, sk*bg] per-(group, slab) blocks."""
    in_f, batch = xT.shape
    ns = in_f // P // sk
    ng = batch // bg
    return np.ascontiguousarray(
        xT.reshape(ns, sk, P, ng, bg).transpose(3, 0, 2, 1, 4)
        .reshape(ng * ns * P, sk * bg))


def unslab_y(yS, o_shard, batch, bg=BG):
    """[ng*P, oc*bg] device layout -> [o_shard, batch] y^T block."""
    ng = batch // bg
    oc = o_shard // P
    return (yS.reshape(ng, P, oc, bg).transpose(2, 1, 0, 3)
            .reshape(o_shard, batch))


def build_nc(batch=B_SHARD, in_f=IN_F, o_shard=O_SHARD, bg=BG, sk=SK,
             head=HEAD, xsk=XSK):
    import concourse.mybir as mybir
    from concourse import bacc
    from concourse.tile import TileContext

    p = P
    kt = in_f // p          # k tiles along contraction
    sizes = wm_slab_sizes(kt, head, sk)
    nsm = (kt - head) // sk  # main weight slabs
    nsx = kt // xsk          # x slabs per group
    oc = o_shard // p        # out-feature chunks of 128
    ng = batch // bg         # batch groups
    oh = (oc + 1) // 2       # chunks per engine half
    bf16 = mybir.dt.bfloat16
    f32 = mybir.dt.float32

    nc = bacc.Bacc()
    xS = nc.declare_dram_parameter("xS", [ng * nsx * p, xsk * bg], bf16,
                                   isOutput=False)
    wH = nc.declare_dram_parameter("wH", [head * p, o_shard], bf16,
                                   isOutput=False)
    wM = nc.declare_dram_parameter("wM", [nsm * p, sk * o_shard], bf16,
                                   isOutput=False)
    bT = nc.declare_dram_parameter("bT", [p, oc], f32, isOutput=False)
    yS = nc.declare_dram_parameter("yS", [ng * p, oc * bg], bf16,
                                   isOutput=True)

    xv = xS[:].rearrange("(g s p) w -> g s p w", s=nsx, p=p)
    wHv = wH[:].rearrange("(s p) w -> s p w", p=p)
    wMv = wM[:].rearrange("(s p) w -> s p w", p=p)
    yv = yS[:].rearrange("(g p) w -> g p w", p=p)

    # slab s -> (view, local idx, width, first k-tile)
    slabs = []
    k0 = 0
    for s, sz in enumerate(sizes):
        slabs.append((wHv if s < head else wMv,
                      s if s < head else s - head, sz, k0))
        k0 += sz

    with TileContext(nc) as tc:
        with tc.tile_pool(name="const", bufs=1) as cpool, \
             tc.tile_pool(name="xin", bufs=8) as xpool, \
             tc.tile_pool(name="acc", bufs=8, space="PSUM") as ppool, \
             tc.tile_pool(name="out", bufs=2) as opool:

            # PE warmup: burn the DVFS ramp on zeros while DMAs arrive
            warm = cpool.tile([p, bg], bf16, tag="warm")
            nc.vector.memset(warm, 0.0)
            wps = ppool.tile([p, bg], f32, tag="ps", name="warm")
            for _ in range(N_WARM):
                nc.tensor.matmul(wps, warm[:, :p], warm,
                                 start=True, stop=True)

            # ignition: k-tiles 0..head-1 arrive as half-width weight
            # tiles and single-k x tiles (128KB DMAs), so the first
            # matmuls fire as early as possible after the DMA engines
            # spin up.
            ohw = oh * p
            xi, wi = [], []
            for ki in range(head):
                xt = xpool.tile([p, bg], bf16, tag=f"xi{ki}")
                nc.scalar.dma_start(
                    out=xt, in_=xv[0, ki // xsk][:, (ki % xsk) * bg:
                                                 (ki % xsk + 1) * bg])
                xi.append(xt)
                row = []
                for hh in range(2):
                    lo, hi = hh * ohw, min((hh + 1) * ohw, o_shard)
                    t = cpool.tile([p, hi - lo], bf16, tag=f"wi{ki}_{hh}")
                    nc.sync.dma_start(out=t, in_=wHv[ki][:, lo:hi])
                    row.append(t)
                wi.append(row)

            xtiles = {}

            def issue_x(g, s, eng=None):
                t = xpool.tile([p, xsk * bg], bf16, tag="x",
                               name=f"x{g}_{s}")
                (eng or nc.scalar).dma_start(out=t, in_=xv[g, s])
                xtiles[(g, s)] = t

            # first two x blocks race the head weight slabs on the
            # Activation queue; the rest of group 0's x interleaves
            # into the SP queue behind the weight slabs that gate the
            # PE, so the weight stream gets the pipe first.
            for s in range(min(2, nsx)):
                issue_x(0, s)

            # masked weight slabs DMA straight into their resident
            # SBUF tiles; fine-grained tiles let matmuls start on
            # slab 0 while later slabs still stream in.  Slab 0 is
            # covered by the ignition tiles and skipped.
            mws = []
            bias_t = None
            for wvv, si, sz, _ in slabs:
                s_idx = len(mws)
                if s_idx < head:
                    mws.append(None)
                    continue
                mw = cpool.tile([p, sz * o_shard], bf16, tag=f"mw{s_idx}")
                nc.sync.dma_start(out=mw, in_=wvv[si])
                mws.append(mw)
                if bias_t is None and s_idx >= 4:
                    bias_t = cpool.tile([p, oc], f32)
                    nc.sync.dma_start(out=bias_t, in_=bT[:])
                xs0 = s_idx
                if head <= s_idx and 2 <= xs0 < nsx:
                    issue_x(0, xs0, eng=nc.sync)
            if bias_t is None:
                bias_t = cpool.tile([p, oc], f32)
                nc.sync.dma_start(out=bias_t, in_=bT[:])
            for s in range(2, nsx):
                if (0, s) not in xtiles:
                    issue_x(0, s)

            for g in range(ng):
                psums = [ppool.tile([p, bg], f32, tag="ps",
                                    name=f"ps{g}_{j}")
                         for j in range(oc)]
                for s, (_, _, sz, k0) in enumerate(slabs):
                    for kk in range(sz):
                        k = k0 + kk
                        if g + 1 < ng and k % (kt // nsx) == 0:
                            issue_x(g + 1, k // (kt // nsx))
                        sx, kx = divmod(k, xsk)
                        if k < head and g == 0:
                            rhs = xi[k]
                        else:
                            rhs = xtiles[(g, sx)][:, kx * bg:(kx + 1) * bg]
                        for j in range(oc):
                            if k < head:
                                lhsT = wi[k][j // oh][:, (j % oh) * p:
                                                      (j % oh + 1) * p]
                            else:
                                lhsT = mws[s][:, kk * o_shard + j * p:
                                              kk * o_shard + (j + 1) * p]
                            nc.tensor.matmul(
                                psums[j], lhsT, rhs,
                                start=(k == 0),
                                stop=(k == kt - 1),
                            )
                        if (kx == xsk - 1 or k == kt - 1) and \
                                not (g == 0 and k < head):
                            xtiles.pop((g, sx))
                # evacuate: DVE handles the first half-group (DMAs on
                # the SP queue), the Activation engine the second
                # (DMAs on its own queue), in quarter-group DMAs so
                # issue+transfer overlap the evac chains.  The last
                # group's final quarter is split per-chunk across both
                # queues to shorten the drain after the last matmul.
                oq = max(oh // 2, 1)
                quarters = []
                for h in range(4):
                    lo, hi = h * oq, min((h + 1) * oq, oc)
                    if lo < hi:
                        quarters.append((lo, hi))
                if g == ng - 1 and quarters[-1][1] - quarters[-1][0] > 1:
                    lo, hi = quarters.pop()
                    quarters += [(jj, jj + 1) for jj in range(lo, hi)]
                for h, (lo, hi) in enumerate(quarters):
                    ot = opool.tile([p, (hi - lo) * bg], bf16,
                                    tag=f"o{h}_{hi - lo}", name=f"o{g}_{h}")
                    for j in range(lo, hi):
                        dst = ot[:, (j - lo) * bg:(j - lo + 1) * bg]
                        if j < oh:
                            nc.vector.tensor_scalar_add(
                                out=dst, in0=psums[j],
                                scalar1=bias_t[:, j:j + 1])
                        else:
                            nc.scalar.activation(
                                out=dst, in_=psums[j],
                                func=mybir.ActivationFunctionType.Identity,
                                bias=bias_t[:, j:j + 1], scale=1.0)
                    if g == ng - 1 and hi - lo == 1 and lo >= oh:
                        # last group's split chunks alternate queues
                        eng = nc.scalar if lo % 2 == 0 else nc.sync
                    else:
                        eng = nc.sync if lo < oh else nc.scalar
                    eng.dma_start(
                        out=yv[g, :, lo * bg:hi * bg], in_=ot)
    nc.finalize()
    return nc


def _prep_in_maps(x, weight, bias, myFilter):
    oc = O_SHARD // P
    hk = HEAD * P
    xb = np.asarray(x, np.float32).astype(_BF16)
    xT = xb.T
    xSb = [slab_x(np.ascontiguousarray(xT[:, b * B_SHARD:(b + 1) * B_SHARD]))
           for b in range(N_BB)]
    w_masked = np.asarray(weight, np.float32) * np.asarray(
        myFilter, np.float32)
    per_ob = []
    for ob in range(N_OB):
        rows = slice(ob * O_SHARD, (ob + 1) * O_SHARD)
        wT = w_masked[rows].T.astype(_BF16)
        per_ob.append({
            "wH": slab_weights(wT[:hk], 1),
            "wM": slab_weights(wT[hk:], SK),
            "bT": np.ascontiguousarray(
                np.asarray(bias, np.float32)[rows].reshape(oc, P).T),
        })
    in_maps = []
    for c in range(N_CORES):
        ob, bb = c // N_BB, c % N_BB
        m = dict(per_ob[ob])
        m["xS"] = xSb[bb]
        in_maps.append(m)
    return in_maps


def kernel(x, weight, bias, myFilter):
    global _NC, LAST_RESULT
    _ensure_axon_hooks_stub()
    from concourse.bass_utils import run_bass_kernel_spmd

    if _NC is None:
        _NC = build_nc()

    in_maps = _prep_in_maps(x, weight, bias, myFilter)

    kwargs = {}
    if os.environ.get("KERNEL_TRACE") == "1":
        _install_real_ntff_hook()
        kwargs["trace"] = True
        tdir = os.environ.get("KERNEL_TRACE_DIR")
        if tdir:
            kwargs["tmpdir"] = tdir

    res = run_bass_kernel_spmd(_NC, in_maps, list(range(N_CORES)), **kwargs)
    LAST_RESULT = res

    y = np.empty((BATCH, OUT_F), dtype=np.float32)
    for c in range(N_CORES):
        ob, bb = c // N_BB, c % N_BB
        yT_blk = unslab_y(np.asarray(res.results[c]["yS"]), O_SHARD, B_SHARD)
        y[bb * B_SHARD:(bb + 1) * B_SHARD,
          ob * O_SHARD:(ob + 1) * O_SHARD] = yT_blk.T
    return y


# revision 26
# speedup vs baseline: 1.0114x; 1.0114x over previous
"""Locally-connected (masked linear) layer for 8 TRN2 NeuronCores.

y = x @ (W * M)^T + b
  x: [4096, 4096] f32, W/M: [4096, 4096] f32, b: [4096] f32.

Strategy (2D: 4-way over out_features x 2-way over batch):
  - Core c owns out rows [ob*1024, (ob+1)*1024) (ob = c>>1) and batch
    rows [bb*2048, (bb+1)*2048) (bb = c&1).  This halves the x stream
    per batch group vs pure tensor-parallel, so weight streaming
    during the first group fits far under the HBM budget.
  - M is a static binary neighborhood filter, so the host folds it
    into the weights while laying them out (masked_w = W*M in bf16,
    exact since M is {0,1}); the device streams masked weight slabs
    straight into their resident SBUF tiles.
  - Host uploads contraction-major slabs: two 1-k-tile head weight
    slabs (so the PE can start early) then 2-k-tile slabs
    ([128, 2*1024] bf16, 4KB contiguous rows), and x in 4-k-tile
    blocks [128, 4*512] per batch group.  Weights issue on the SP
    queue; x rides the Activation queue except group-0 blocks >=2,
    which interleave into the SP queue behind the weight slabs that
    gate the PE.  Large contiguous DMAs keep the ~650ns/DMA
    descriptor-generation cost off the critical path.
  - A handful of warmup matmuls on a zeroed tile burn the PE DVFS
    ramp during the initial DMA wait so real matmuls start at full
    clock.
  - PE: y^T[j] += mw[k]^T.T @ x[k] accumulated over 32 k-tiles in 8
    fp32 PSUM banks per batch group; with oc=8 the next group's bank
    reuse hides under the current group's tail.
  - Evacuation adds bias per-partition and casts to bf16, split
    between DVE (j 0-3) and the Activation engine (j 4-7) in
    quarter-group DMAs so the last group's transfers overlap the
    evac chain.
  - Host reassembles the 8 y^T blocks, transposes, casts to f32.
"""

import os

import numpy as np
import ml_dtypes

BATCH = 4096
IN_F = 4096
OUT_F = 4096
N_CORES = 8
N_OB = 4                    # out-feature blocks
N_BB = 2                    # batch blocks
O_SHARD = OUT_F // N_OB     # 1024 out rows per core
B_SHARD = BATCH // N_BB     # 2048 batch rows per core
P = 128                     # SBUF partitions
BG = 512                    # batch columns per PSUM accumulation group
SK = 2                      # k-tiles per main weight slab
HEAD = 2                    # leading 1-k-tile weight slabs
XSK = 4                     # k-tiles per x slab
N_WARM = 7                  # PE warmup matmuls

_BF16 = ml_dtypes.bfloat16
_NC = None
LAST_RESULT = None


def _ensure_axon_hooks_stub():
    """bass_utils' axon trace path imports antenv.axon_hooks, which this
    container's antenv stub lacks. Install a minimal registry so the
    import succeeds (hook None => bass_utils skips tracing gracefully)."""
    import sys
    import types

    try:
        import antenv.axon_hooks  # noqa: F401
        return
    except ImportError:
        pass
    import antenv

    mod = types.ModuleType("antenv.axon_hooks")
    mod._HOOK = None

    def set_axon_ntff_profile_hook(h):
        mod._HOOK = h

    def get_axon_ntff_profile_hook():
        return mod._HOOK

    mod.set_axon_ntff_profile_hook = set_axon_ntff_profile_hook
    mod.get_axon_ntff_profile_hook = get_axon_ntff_profile_hook
    antenv.axon_hooks = mod
    sys.modules["antenv.axon_hooks"] = mod


def _install_real_ntff_hook():
    """Wire the ctypes NTFF profiling hook (normally registered by the
    boot middleware) so run_bass_kernel_spmd(trace=True) works."""
    _ensure_axon_hooks_stub()
    import antenv.axon_hooks as ah

    if ah.get_axon_ntff_profile_hook() is None:
        try:
            from trn_agent_boot.trn_boot import _ntff_profile_via_ctypes

            hook = _ntff_profile_via_ctypes("/opt/axon/libaxon_pjrt.so")
            if hook is not None:
                ah.set_axon_ntff_profile_hook(hook)
        except Exception:
            pass
    try:
        import concourse.bass_utils as bu

        bu.upload_artifacts = lambda tmpdir: "local://" + str(tmpdir)
    except Exception:
        pass


def wm_slab_sizes(kt, head=HEAD, sk=SK):
    """k-tile counts per weight slab: HEAD leading singles, then SK-wide."""
    assert kt > head and (kt - head) % sk == 0
    return [1] * head + [sk] * ((kt - head) // sk)


def slab_weights(wT, sk):
    """[k*P, o] contraction-major -> [nslab*P, sk*o] slab layout."""
    kp, o = wT.shape
    ns = kp // P // sk
    return np.ascontiguousarray(
        wT.reshape(ns, sk, P, o).transpose(0, 2, 1, 3).reshape(ns * P, sk * o))


def slab_x(xT, bg=BG, sk=XSK):
    """[in_f, batch] -> [ng*nslab*P, sk*bg] per-(group, slab) blocks."""
    in_f, batch = xT.shape
    ns = in_f // P // sk
    ng = batch // bg
    return np.ascontiguousarray(
        xT.reshape(ns, sk, P, ng, bg).transpose(3, 0, 2, 1, 4)
        .reshape(ng * ns * P, sk * bg))


def unslab_y(yS, o_shard, batch, bg=BG):
    """[ng*P, oc*bg] device layout -> [o_shard, batch] y^T block."""
    ng = batch // bg
    oc = o_shard // P
    return (yS.reshape(ng, P, oc, bg).transpose(2, 1, 0, 3)
            .reshape(o_shard, batch))


def build_nc(batch=B_SHARD, in_f=IN_F, o_shard=O_SHARD, bg=BG, sk=SK,
             head=HEAD, xsk=XSK):
    import concourse.mybir as mybir
    from concourse import bacc
    from concourse.tile import TileContext

    p = P
    kt = in_f // p          # k tiles along contraction
    sizes = wm_slab_sizes(kt, head, sk)
    nsm = (kt - head) // sk  # main weight slabs
    nsx = kt // xsk          # x slabs per group
    oc = o_shard // p        # out-feature chunks of 128
    ng = batch // bg         # batch groups
    oh = (oc + 1) // 2       # chunks per engine half
    bf16 = mybir.dt.bfloat16
    f32 = mybir.dt.float32

    nc = bacc.Bacc()
    xS = nc.declare_dram_parameter("xS", [ng * nsx * p, xsk * bg], bf16,
                                   isOutput=False)
    wH = nc.declare_dram_parameter("wH", [head * p, o_shard], bf16,
                                   isOutput=False)
    wM = nc.declare_dram_parameter("wM", [nsm * p, sk * o_shard], bf16,
                                   isOutput=False)
    bT = nc.declare_dram_parameter("bT", [p, oc], f32, isOutput=False)
    yS = nc.declare_dram_parameter("yS", [ng * p, oc * bg], bf16,
                                   isOutput=True)

    xv = xS[:].rearrange("(g s p) w -> g s p w", s=nsx, p=p)
    wHv = wH[:].rearrange("(s p) w -> s p w", p=p)
    wMv = wM[:].rearrange("(s p) w -> s p w", p=p)
    yv = yS[:].rearrange("(g p) w -> g p w", p=p)

    # slab s -> (view, local idx, width, first k-tile)
    slabs = []
    k0 = 0
    for s, sz in enumerate(sizes):
        slabs.append((wHv if s < head else wMv,
                      s if s < head else s - head, sz, k0))
        k0 += sz

    with TileContext(nc) as tc:
        with tc.tile_pool(name="const", bufs=1) as cpool, \
             tc.tile_pool(name="xin", bufs=8) as xpool, \
             tc.tile_pool(name="acc", bufs=8, space="PSUM") as ppool, \
             tc.tile_pool(name="out", bufs=2) as opool:

            # PE warmup: burn the DVFS ramp on zeros while DMAs arrive
            warm = cpool.tile([p, bg], bf16, tag="warm")
            nc.vector.memset(warm, 0.0)
            wps = ppool.tile([p, bg], f32, tag="ps", name="warm")
            for _ in range(N_WARM):
                nc.tensor.matmul(wps, warm[:, :p], warm,
                                 start=True, stop=True)

            # ignition: k-tile 0 arrives as half-width weight tiles
            # and a single-k x tile (128KB DMAs), so the first
            # matmuls fire as early as possible after the DMA engines
            # spin up.
            ign = 1
            ohw = oh * p
            xi, wi = [], []
            for ki in range(ign):
                xt = xpool.tile([p, bg], bf16, tag=f"xi{ki}")
                nc.scalar.dma_start(
                    out=xt, in_=xv[0, ki // xsk][:, (ki % xsk) * bg:
                                                 (ki % xsk + 1) * bg])
                xi.append(xt)
                row = []
                for hh in range(2):
                    lo, hi = hh * ohw, min((hh + 1) * ohw, o_shard)
                    t = cpool.tile([p, hi - lo], bf16, tag=f"wi{ki}_{hh}")
                    nc.sync.dma_start(out=t, in_=wHv[ki][:, lo:hi])
                    row.append(t)
                wi.append(row)

            xtiles = {}

            def issue_x(g, s, eng=None):
                t = xpool.tile([p, xsk * bg], bf16, tag="x",
                               name=f"x{g}_{s}")
                (eng or nc.scalar).dma_start(out=t, in_=xv[g, s])
                xtiles[(g, s)] = t

            # first two x blocks race the head weight slabs on the
            # Activation queue; the rest of group 0's x interleaves
            # into the SP queue behind the weight slabs that gate the
            # PE, so the weight stream gets the pipe first.
            for s in range(min(2, nsx)):
                issue_x(0, s)

            # masked weight slabs DMA straight into their resident
            # SBUF tiles; fine-grained tiles let matmuls start on
            # slab 0 while later slabs still stream in.  Slab 0 is
            # covered by the ignition tiles and skipped.
            mws = []
            bias_t = None
            for wvv, si, sz, _ in slabs:
                s_idx = len(mws)
                if s_idx < ign:
                    mws.append(None)
                    continue
                mw = cpool.tile([p, sz * o_shard], bf16, tag=f"mw{s_idx}")
                nc.sync.dma_start(out=mw, in_=wvv[si])
                mws.append(mw)
                if bias_t is None and s_idx >= 4:
                    bias_t = cpool.tile([p, oc], f32)
                    nc.sync.dma_start(out=bias_t, in_=bT[:])
                xs0 = s_idx
                if head <= s_idx and 2 <= xs0 < nsx:
                    issue_x(0, xs0, eng=nc.sync)
            if bias_t is None:
                bias_t = cpool.tile([p, oc], f32)
                nc.sync.dma_start(out=bias_t, in_=bT[:])
            for s in range(2, nsx):
                if (0, s) not in xtiles:
                    issue_x(0, s)

            for g in range(ng):
                psums = [ppool.tile([p, bg], f32, tag="ps",
                                    name=f"ps{g}_{j}")
                         for j in range(oc)]
                for s, (_, _, sz, k0) in enumerate(slabs):
                    for kk in range(sz):
                        k = k0 + kk
                        if g + 1 < ng and k % (kt // nsx) == 0:
                            issue_x(g + 1, k // (kt // nsx))
                        sx, kx = divmod(k, xsk)
                        if k < ign and g == 0:
                            rhs = xi[k]
                        else:
                            rhs = xtiles[(g, sx)][:, kx * bg:(kx + 1) * bg]
                        for j in range(oc):
                            if k < ign:
                                lhsT = wi[k][j // oh][:, (j % oh) * p:
                                                      (j % oh + 1) * p]
                            else:
                                lhsT = mws[s][:, kk * o_shard + j * p:
                                              kk * o_shard + (j + 1) * p]
                            nc.tensor.matmul(
                                psums[j], lhsT, rhs,
                                start=(k == 0),
                                stop=(k == kt - 1),
                            )
                        if (kx == xsk - 1 or k == kt - 1) and \
                                not (g == 0 and k < ign):
                            xtiles.pop((g, sx))
                # evacuate: DVE handles the first half-group (DMAs on
                # the SP queue), the Activation engine the second
                # (DMAs on its own queue), in quarter-group DMAs so
                # issue+transfer overlap the evac chains.  The last
                # group's final quarter is split per-chunk across both
                # queues to shorten the drain after the last matmul.
                oq = max(oh // 2, 1)
                quarters = []
                for h in range(4):
                    lo, hi = h * oq, min((h + 1) * oq, oc)
                    if lo < hi:
                        quarters.append((lo, hi))
                if g == ng - 1 and quarters[-1][1] - quarters[-1][0] > 1:
                    lo, hi = quarters.pop()
                    quarters += [(jj, jj + 1) for jj in range(lo, hi)]
                for h, (lo, hi) in enumerate(quarters):
                    ot = opool.tile([p, (hi - lo) * bg], bf16,
                                    tag=f"o{h}_{hi - lo}", name=f"o{g}_{h}")
                    for j in range(lo, hi):
                        dst = ot[:, (j - lo) * bg:(j - lo + 1) * bg]
                        if j < oh:
                            nc.vector.tensor_scalar_add(
                                out=dst, in0=psums[j],
                                scalar1=bias_t[:, j:j + 1])
                        else:
                            nc.scalar.activation(
                                out=dst, in_=psums[j],
                                func=mybir.ActivationFunctionType.Identity,
                                bias=bias_t[:, j:j + 1], scale=1.0)
                    if g == ng - 1 and hi - lo == 1 and lo >= oh:
                        # last group's split chunks alternate queues
                        eng = nc.scalar if lo % 2 == 0 else nc.sync
                    else:
                        eng = nc.sync if lo < oh else nc.scalar
                    eng.dma_start(
                        out=yv[g, :, lo * bg:hi * bg], in_=ot)
    nc.finalize()
    return nc


def _prep_in_maps(x, weight, bias, myFilter):
    oc = O_SHARD // P
    hk = HEAD * P
    xb = np.asarray(x, np.float32).astype(_BF16)
    xT = xb.T
    xSb = [slab_x(np.ascontiguousarray(xT[:, b * B_SHARD:(b + 1) * B_SHARD]))
           for b in range(N_BB)]
    w_masked = np.asarray(weight, np.float32) * np.asarray(
        myFilter, np.float32)
    per_ob = []
    for ob in range(N_OB):
        rows = slice(ob * O_SHARD, (ob + 1) * O_SHARD)
        wT = w_masked[rows].T.astype(_BF16)
        per_ob.append({
            "wH": slab_weights(wT[:hk], 1),
            "wM": slab_weights(wT[hk:], SK),
            "bT": np.ascontiguousarray(
                np.asarray(bias, np.float32)[rows].reshape(oc, P).T),
        })
    in_maps = []
    for c in range(N_CORES):
        ob, bb = c // N_BB, c % N_BB
        m = dict(per_ob[ob])
        m["xS"] = xSb[bb]
        in_maps.append(m)
    return in_maps


def kernel(x, weight, bias, myFilter):
    global _NC, LAST_RESULT
    _ensure_axon_hooks_stub()
    from concourse.bass_utils import run_bass_kernel_spmd

    if _NC is None:
        _NC = build_nc()

    in_maps = _prep_in_maps(x, weight, bias, myFilter)

    kwargs = {}
    if os.environ.get("KERNEL_TRACE") == "1":
        _install_real_ntff_hook()
        kwargs["trace"] = True
        tdir = os.environ.get("KERNEL_TRACE_DIR")
        if tdir:
            kwargs["tmpdir"] = tdir

    res = run_bass_kernel_spmd(_NC, in_maps, list(range(N_CORES)), **kwargs)
    LAST_RESULT = res

    y = np.empty((BATCH, OUT_F), dtype=np.float32)
    for c in range(N_CORES):
        ob, bb = c // N_BB, c % N_BB
        yT_blk = unslab_y(np.asarray(res.results[c]["yS"]), O_SHARD, B_SHARD)
        y[bb * B_SHARD:(bb + 1) * B_SHARD,
          ob * O_SHARD:(ob + 1) * O_SHARD] = yT_blk.T
    return y
